# revision 8
# baseline (speedup 1.0000x reference)
"""LIF spiking layer (T=32, B=256, C_in=C_out=4096, fp32) on 8 trn2 NeuronCores.

Strategy: data-parallel over batch (32 samples/core, W replicated).
Host-side numpy pre-permutes both operands into SBUF tile layout (contraction
dim ci on partitions), so each core only runs matmuls + the recurrence:
  current[co, (t,b)] = W @ x_core.T  on TensorE per 128-co tile (psum),
  LIF membrane recurrence over t on VectorE with mem laid out [co=128, b=32],
  spikes stored [co, (t,b)] and transposed back on the host.

MODE "fp32" is bit-exact vs the fp32 jax reference; "fp16x3" computes the
matmul as three fp16 hi/lo passes (25% faster, ~9e-4 rel err).
"""

import os

import numpy as np

import concourse.mybir as mybir
import concourse.tile as tile
from concourse import bacc
from concourse.bass_utils import run_bass_kernel_spmd

FP32 = mybir.dt.float32
FP16 = mybir.dt.float16

N_CORES = 8
T, B, CI, CO = 32, 256, 4096, 4096
B_LOC = B // N_CORES  # 32
TB = T * B_LOC  # 1024
# Exact powers of 2; the LIF recurrence is exactly scale-equivariant, and
# scaling keeps the fp16 lo-components out of subnormal range on the PE.
WSCALE = 64.0
XSCALE = 128.0
SCALE = WSCALE * XSCALE

# set by test.py to collect a profile
TRACE = False
LAST_EXEC_NS = None
# "hi16lo8": exact fp16 hi-pass + one fp8e4 DoubleRow correction pass
# (1.5 PE cycles/row, matmul rel err ~1e-5).
# "fp16x3": 3-pass fp16 hi/lo split matmul — rel err ~9e-4 (4 of 33.5M spikes
# flip), robust across ~70 device runs.
# "fp32": bit-exact vs the fp32 jax reference (0 mismatches) but native-fp32
# matmul streams intermittently wedge the exec unit on this hardware
# (NRT_EXEC_UNIT_UNRECOVERABLE in 2 of 5 runs), so it is not the default.
MODE = os.environ.get("LIF_KERNEL_MODE", "hi16lo8")

_CACHE = {}


def build_kernel_hi16lo8(
    d: float,
    th: float,
    has_bias: bool,
    T=T,
    B_loc=B_LOC,
    CI=CI,
    CO=CO,
):
    """2-pass matmul kernel: exact fp16 hi-pass + one fp8e4 DoubleRow pass
    that captures both cross terms via K-stacking (1.5 PE cycles/row total,
    matmul rel err ~1e-5):

      pass1: fp16(W*2^6).T @ fp16(x*2^11)                     (k=128/matmul)
      pass2: DoubleRow fp8 [Wl*2^17; W11*2^6] @ [x; xl*2^11]  (k=256/matmul)

    Both accumulate into one PSUM group at scale 2^17; the Activation-engine
    PSUM->SBUF drain applies 2^-17.

    Schedule: X pieces (96KB/partition fp16+fp8) stay resident in SBUF; W
    streams as 32 co-tiles (fp16+fp8, triple-buffered). co-tiles are
    processed in pairs ("groups"): PE accumulates both tiles' currents into
    PSUM, the Activation engine drains PSUM chunks into SBUF staging, and
    DVE runs the membrane recurrence for both tiles at once ([128, 64] per
    step), writing spikes in-place over the staged currents. Group g's
    recurrence overlaps group g+1's matmuls; spikes DMA out from the Pool
    engine (SWDGE) so the SP W-stream is never blocked. The first group
    consumes X k-tiles in DMA arrival order so the PE starts ~4us into the
    X stream.
    """
    FP8 = mybir.dt.float8e4
    TBl = T * B_loc
    n_k = CI // 128
    n_c = CO // 128
    csize = 512
    n_chunk = TBl // csize
    G = 2  # co-tiles per recurrence group
    n_g = n_c // G
    OSCALE = 2.0**-17

    nc = bacc.Bacc("TRN2", target_bir_lowering=False, debug=False, num_devices=N_CORES)

    x16 = nc.declare_dram_parameter("x16", [128, n_k, TBl], FP16, isOutput=False)
    x8 = nc.declare_dram_parameter("x8", [128, n_k, 2, TBl], FP8, isOutput=False)
    w16 = nc.declare_dram_parameter("w16", [n_c, 128, n_k, 128], FP16, isOutput=False)
    w8 = nc.declare_dram_parameter("w8", [n_c, 128, n_k, 2, 128], FP8, isOutput=False)
    if has_bias:
        bias = nc.declare_dram_parameter("bias", [CO, 1], FP32, isOutput=False)
    spkT = nc.declare_dram_parameter("spkT", [CO, TBl], FP32, isOutput=True)

    with tile.TileContext(nc) as tc:
        with (
            tc.tile_pool(name="xtp", bufs=1) as xt_pool,
            tc.tile_pool(name="wtp", bufs=3) as wt_pool,
            tc.tile_pool(name="stg", bufs=2) as stg_pool,
            tc.tile_pool(name="small", bufs=1) as small_pool,
            tc.tile_pool(name="pc", bufs=8, space="PSUM") as pc_pool,
        ):
            X16 = xt_pool.tile([128, n_k, TBl], FP16)
            X8 = xt_pool.tile([128, n_k, 2, TBl], FP8)
            mem = small_pool.tile([128, G * B_loc], FP32)

            # --- prologue DMA order (all on the SP queue): the first two W
            # tiles stream in k-strips interleaved with the first X k-tiles
            # so group-0 matmuls start a few us in; the X bulk follows; later
            # W tiles ride behind (paced by wt-pool buffer reuse).
            W16s = [None] * n_c
            W8s = [None] * n_c

            def w_tiles(c, nm):
                W16s[c] = wt_pool.tile(
                    [128, n_k, 128], FP16, tag="w16", name=f"w16_{nm}"
                )
                W8s[c] = wt_pool.tile(
                    [128, n_k, 2, 128], FP8, tag="w8", name=f"w8_{nm}"
                )

            def w_dma(c):
                nc.sync.dma_start(out=W16s[c], in_=w16[c, :, :, :])
                nc.sync.dma_start(out=W8s[c], in_=w8[c, :, :, :, :])

            for c in (0, 1):
                w_tiles(c, str(c))
            wq = 8  # k-slices per W strip

            def w_strip(c, sidx):
                ks = slice(sidx * wq, (sidx + 1) * wq)
                nc.sync.dma_start(out=W16s[c][:, ks, :], in_=w16[c, :, ks, :])
                nc.sync.dma_start(out=W8s[c][:, ks, :, :], in_=w8[c, :, ks, :, :])

            def x_k(k):
                nc.sync.dma_start(out=X16[:, k, :], in_=x16[:, k, :])
                nc.sync.dma_start(out=X8[:, k, :, :], in_=x8[:, k, :, :])

            # strip s of W0/W1 is needed at matmul k=8s; spread strips over
            # the X stream so X k-tiles arrive as early as possible
            w_strip(0, 0)
            w_strip(1, 0)
            for k in range(n_k):
                x_k(k)
                if k in (3, 7, 11):
                    sidx = (k + 1) // 4
                    w_strip(0, sidx)
                    w_strip(1, sidx)
            # W2 right after the X bulk so tile 2 can start after group 0
            w_tiles(2, "2")
            w_dma(2)
            for c in range(3, n_c):
                w_tiles(c, "c")
                w_dma(c)
            if has_bias:
                b_tile = small_pool.tile([CO // 128, 128, 1], FP32)
                nc.sync.dma_start(
                    out=b_tile, in_=bias[:, :].reshape([CO // 128, 128, 1])
                )

            def mm16(pc, c, k, ch, start):
                nc.tensor.matmul(
                    pc,
                    lhsT=W16s[c][:, k, :],
                    rhs=X16[:, k, ch * csize : (ch + 1) * csize],
                    start=start,
                    stop=False,
                    skip_group_check=True,
                )

            def mm8(pc, c, k, ch, stop):
                nc.tensor.matmul(
                    pc,
                    lhsT=W8s[c][:, k, :, :],
                    rhs=X8[:, k, :, ch * csize : (ch + 1) * csize],
                    start=False,
                    stop=stop,
                    perf_mode=mybir.MatmulPerfMode.DoubleRow,
                    skip_group_check=True,
                )

            def drain(stage, pc, ti, ch):
                nc.scalar.activation(
                    stage[:, ti, ch * csize : (ch + 1) * csize],
                    pc,
                    mybir.ActivationFunctionType.Copy,
                    scale=OSCALE,
                )

            for g in range(n_g):
                c0 = g * G
                stage = stg_pool.tile([128, G, TBl], FP32, tag="stage")
                pcs = [
                    [
                        pc_pool.tile([128, csize], FP32, tag="pc", name="pc")
                        for _ in range(n_chunk)
                    ]
                    for _ in range(G)
                ]
                if g == 0:
                    # k-outer: consume X k-tiles in DMA arrival order
                    for k in range(n_k):
                        for ti in range(G):
                            for ch in range(n_chunk):
                                mm16(pcs[ti][ch], c0 + ti, k, ch, k == 0)
                        for ti in range(G):
                            for ch in range(n_chunk):
                                mm8(pcs[ti][ch], c0 + ti, k, ch, k == n_k - 1)
                    for ti in range(G):
                        for ch in range(n_chunk):
                            drain(stage, pcs[ti][ch], ti, ch)
                else:
                    # ch-outer: both tiles' chunk-0 currents are staged by
                    # mid-group, so the recurrence's first half overlaps the
                    # group's second-half matmuls
                    for ch in range(n_chunk):
                        for ti in range(G):
                            for k in range(n_k):
                                mm16(pcs[ti][ch], c0 + ti, k, ch, k == 0)
                            for k in range(n_k):
                                mm8(pcs[ti][ch], c0 + ti, k, ch, k == n_k - 1)
                            drain(stage, pcs[ti][ch], ti, ch)

                # LIF recurrence for the G tiles at once, spikes written
                # in-place over the staged currents
                nc.vector.memset(mem, 0.0)
                for t in range(T):
                    o = t * B_loc
                    cur = stage[:, :, o : o + B_loc]
                    nc.vector.scalar_tensor_tensor(
                        out=mem,
                        in0=mem,
                        scalar=d,
                        in1=cur,
                        op0=mybir.AluOpType.mult,
                        op1=mybir.AluOpType.add,
                    )
                    if has_bias:
                        for ti in range(G):
                            nc.vector.tensor_scalar(
                                mem[:, ti * B_loc : (ti + 1) * B_loc],
                                mem[:, ti * B_loc : (ti + 1) * B_loc],
                                b_tile[c0 + ti],
                                None,
                                mybir.AluOpType.add,
                            )
                    nc.vector.tensor_scalar(
                        cur, mem, float(th), None, mybir.AluOpType.is_gt
                    )
                    nc.vector.scalar_tensor_tensor(
                        out=mem,
                        in0=cur,
                        scalar=-float(th),
                        in1=mem,
                        op0=mybir.AluOpType.mult,
                        op1=mybir.AluOpType.add,
                    )
                    if (t + 1) * B_loc % csize == 0:
                        # this chunk's spikes are final: stream them out now
                        cb = (t + 1) * B_loc - csize
                        for ti in range(G):
                            nc.gpsimd.dma_start(
                                out=spkT[
                                    (c0 + ti) * 128 : (c0 + ti + 1) * 128,
                                    cb : cb + csize,
                                ],
                                in_=stage[:, ti, cb : cb + csize],
                            )

    nc.compile()
    return nc


def build_kernel_fp16x3(
    d: float,
    th: float,
    has_bias: bool,
    T=T,
    B_loc=B_LOC,
    CI=CI,
    CO=CO,
):
    """3-pass fp16 hi/lo kernel. All operands arrive from the host already
    split, scaled, and permuted into SBUF tile layout, so the device does
    only matmuls + the recurrence. Spikes leave in [co, tb] layout."""
    TBl = T * B_loc
    n_k = CI // 128
    n_c = CO // 128
    csize = min(512, TBl)
    n_chunk = TBl // csize
    ths = float(th) * SCALE

    nc = bacc.Bacc("TRN2", target_bir_lowering=False, debug=False, num_devices=N_CORES)

    xh = nc.declare_dram_parameter("xh", [128, n_k, TBl], FP16, isOutput=False)
    xl = nc.declare_dram_parameter("xl", [128, n_k, TBl], FP16, isOutput=False)
    wh = nc.declare_dram_parameter("wh", [n_c, 128, n_k, 128], FP16, isOutput=False)
    wl = nc.declare_dram_parameter("wl", [n_c, 128, n_k, 128], FP16, isOutput=False)
    if has_bias:
        bias = nc.declare_dram_parameter("bias", [CO, 1], FP32, isOutput=False)
    spkT = nc.declare_dram_parameter("spkT", [CO, TBl], FP32, isOutput=True)

    with tile.TileContext(nc) as tc:
        with (
            tc.tile_pool(name="xt", bufs=1) as xt_pool,
            tc.tile_pool(name="wt", bufs=2) as wt_pool,
            tc.tile_pool(name="work", bufs=2) as work_pool,
            tc.tile_pool(name="pc", bufs=2 * n_chunk, space="PSUM") as pc_pool,
        ):
            XH = xt_pool.tile([128, n_k, TBl], FP16)
            XL = xt_pool.tile([128, n_k, TBl], FP16)
            # first W strips ahead of the X bulk on the same HWDGE FIFO
            WH_first = wt_pool.tile([128, n_k, 128], FP16, tag="wh")
            WL_first = wt_pool.tile([128, n_k, 128], FP16, tag="wl")
            wq = min(8, n_k)
            for kq in range(0, n_k, wq):
                nc.sync.dma_start(
                    out=WH_first[:, kq : kq + wq, :], in_=wh[0, :, kq : kq + wq, :]
                )
            nc.sync.dma_start(out=WL_first, in_=wl[0, :, :, :])
            for k in range(n_k):
                nc.sync.dma_start(out=XH[:, k, :], in_=xh[:, k, :])
                nc.sync.dma_start(out=XL[:, k, :], in_=xl[:, k, :])

            for c in range(n_c):
                if c == 0:
                    WH_c, WL_c = WH_first, WL_first
                else:
                    WH_c = wt_pool.tile([128, n_k, 128], FP16, tag="wh")
                    WL_c = wt_pool.tile([128, n_k, 128], FP16, tag="wl")
                    nc.sync.dma_start(out=WH_c, in_=wh[c, :, :, :])
                    nc.sync.dma_start(out=WL_c, in_=wl[c, :, :, :])
                if has_bias:
                    b_tile = work_pool.tile([128, 1], FP32, tag="bt")
                    nc.sync.dma_start(
                        out=b_tile, in_=bias[c * 128 : (c + 1) * 128, :]
                    )

                pcs = [
                    pc_pool.tile([128, csize], FP32, tag="pc", name="pc")
                    for _ in range(n_chunk)
                ]
                n_mm = 3 * n_k
                if c == 0:
                    # consume in DMA arrival order: all passes of k before k+1
                    order = [(k, p) for k in range(n_k) for p in (0, 1, 2)]
                else:
                    order = [(k, p) for p in (0, 1, 2) for k in range(n_k)]
                for ch in range(n_chunk):
                    ops = ((WH_c, XH), (WL_c, XH), (WH_c, XL))
                    for i, (k, p) in enumerate(order):
                        Wt, Xt = ops[p]
                        nc.tensor.matmul(
                            pcs[ch],
                            lhsT=Wt[:, k, :],
                            rhs=Xt[:, k, ch * csize : (ch + 1) * csize],
                            start=(i == 0),
                            stop=(i == n_mm - 1),
                        )

                mem = work_pool.tile([128, B_loc], FP32, tag="mem")
                s_stage = work_pool.tile([128, TBl], FP32, tag="s")
                nc.vector.memset(mem, 0.0)
                for t in range(T):
                    o = t * B_loc
                    cur = pcs[o // csize][:, o % csize : o % csize + B_loc]
                    nc.vector.scalar_tensor_tensor(
                        out=mem,
                        in0=mem,
                        scalar=d,
                        in1=cur,
                        op0=mybir.AluOpType.mult,
                        op1=mybir.AluOpType.add,
                    )
                    if has_bias:
                        nc.vector.tensor_scalar(
                            mem, mem, b_tile, None, mybir.AluOpType.add
                        )
                    s_t = s_stage[:, o : o + B_loc]
                    nc.vector.tensor_scalar(
                        s_t, mem, ths, None, mybir.AluOpType.is_gt
                    )
                    nc.vector.scalar_tensor_tensor(
                        out=mem,
                        in0=s_t,
                        scalar=-ths,
                        in1=mem,
                        op0=mybir.AluOpType.mult,
                        op1=mybir.AluOpType.add,
                    )

                nc.sync.dma_start(
                    out=spkT[c * 128 : (c + 1) * 128, :], in_=s_stage
                )

    nc.compile()
    return nc


def build_kernel_fp32hp(
    d: float,
    th: float,
    has_bias: bool,
    T=T,
    B_loc=B_LOC,
    CI=CI,
    CO=CO,
):
    """Exact-fp32 kernel with host-prepped transposed layouts: the device does
    only fp32 matmuls + the recurrence. Spikes leave in [co, tb] layout."""
    TBl = T * B_loc
    n_k = CI // 128
    n_c = CO // 128
    csize = min(512, TBl)
    n_chunk = TBl // csize

    nc = bacc.Bacc("TRN2", target_bir_lowering=False, debug=False, num_devices=N_CORES)

    xt = nc.declare_dram_parameter("xt", [128, n_k, TBl], FP32, isOutput=False)
    wt = nc.declare_dram_parameter("wt", [n_c, 128, n_k, 128], FP32, isOutput=False)
    if has_bias:
        bias = nc.declare_dram_parameter("bias", [CO, 1], FP32, isOutput=False)
    spkT = nc.declare_dram_parameter("spkT", [CO, TBl], FP32, isOutput=True)

    with tile.TileContext(nc) as tc:
        with (
            tc.tile_pool(name="xtp", bufs=1) as xt_pool,
            tc.tile_pool(name="wtp", bufs=3) as wt_pool,
            tc.tile_pool(name="work", bufs=2) as work_pool,
            tc.tile_pool(name="pc", bufs=4 * n_chunk, space="PSUM") as pc_pool,
        ):
            XT = xt_pool.tile([128, n_k, TBl], FP32)
            # first W strip ahead of the XT bulk on the same HWDGE FIFO, in
            # k-chunks, so co-tile 0's first matmuls start almost immediately
            WT_first = wt_pool.tile([128, n_k, 128], FP32, tag="wt")
            wq = min(8, n_k)
            for kq in range(0, n_k, wq):
                nc.sync.dma_start(
                    out=WT_first[:, kq : kq + wq, :], in_=wt[0, :, kq : kq + wq, :]
                )
            # per-k loads so co-tile 0 consumes tiles in DMA arrival order
            for k in range(n_k):
                nc.sync.dma_start(out=XT[:, k, :], in_=xt[:, k, :])

            for c in range(n_c):
                if c == 0:
                    WT_c = WT_first
                else:
                    WT_c = wt_pool.tile([128, n_k, 128], FP32, tag="wt")
                    nc.sync.dma_start(out=WT_c, in_=wt[c, :, :, :])
                if has_bias:
                    b_tile = work_pool.tile([128, 1], FP32, tag="bt")
                    nc.sync.dma_start(
                        out=b_tile, in_=bias[c * 128 : (c + 1) * 128, :]
                    )

                pcs = [
                    pc_pool.tile([128, csize], FP32, tag="pc", name="pc")
                    for _ in range(n_chunk)
                ]
                if c == 0:
                    # k outer: consume XT tiles as they arrive from DRAM
                    for k in range(n_k):
                        for ch in range(n_chunk):
                            nc.tensor.matmul(
                                pcs[ch],
                                lhsT=WT_c[:, k, :],
                                rhs=XT[:, k, ch * csize : (ch + 1) * csize],
                                start=(k == 0),
                                stop=(k == n_k - 1),
                            )
                else:
                    # chunk outer: chunk0 psum frees early for the recurrence
                    for ch in range(n_chunk):
                        for k in range(n_k):
                            nc.tensor.matmul(
                                pcs[ch],
                                lhsT=WT_c[:, k, :],
                                rhs=XT[:, k, ch * csize : (ch + 1) * csize],
                                start=(k == 0),
                                stop=(k == n_k - 1),
                            )

                mem = work_pool.tile([128, B_loc], FP32, tag="mem")
                s_stage = work_pool.tile([128, TBl], FP32, tag="s")
                nc.vector.memset(mem, 0.0)
                for t in range(T):
                    o = t * B_loc
                    cur = pcs[o // csize][:, o % csize : o % csize + B_loc]
                    nc.vector.scalar_tensor_tensor(
                        out=mem,
                        in0=mem,
                        scalar=d,
                        in1=cur,
                        op0=mybir.AluOpType.mult,
                        op1=mybir.AluOpType.add,
                    )
                    if has_bias:
                        nc.vector.tensor_scalar(
                            mem, mem, b_tile, None, mybir.AluOpType.add
                        )
                    s_t = s_stage[:, o : o + B_loc]
                    nc.vector.tensor_scalar(
                        s_t, mem, float(th), None, mybir.AluOpType.is_gt
                    )
                    nc.vector.scalar_tensor_tensor(
                        out=mem,
                        in0=s_t,
                        scalar=-float(th),
                        in1=mem,
                        op0=mybir.AluOpType.mult,
                        op1=mybir.AluOpType.add,
                    )

                nc.sync.dma_start(
                    out=spkT[c * 128 : (c + 1) * 128, :], in_=s_stage
                )

    nc.compile()
    return nc


def _split16(a32):
    hi = a32.astype(np.float16)
    lo = (a32 - hi.astype(np.float32)).astype(np.float16)
    return hi, lo


def _xt_layout(xs):
    """[TB, CI] -> [128, CI//128, TB] so SBUF partition p holds ci = k*128+p."""
    TBl, CIl = xs.shape
    return np.ascontiguousarray(
        xs.reshape(TBl, CIl // 128, 128).transpose(2, 1, 0)
    )


def _wt_layout(Wm):
    """[CO, CI] -> [CO//128, 128, CI//128, 128]: strip c, partition p=ci%128,
    k=ci//128, j=co%128 -> W[c*128+j, k*128+p]."""
    COl, CIl = Wm.shape
    return np.ascontiguousarray(
        Wm.reshape(COl // 128, 128, CIl // 128, 128).transpose(0, 3, 2, 1)
    )


def kernel(x, W, b, decay, thresh):
    global LAST_EXEC_NS
    x = np.ascontiguousarray(np.asarray(x, dtype=np.float32))
    W = np.ascontiguousarray(np.asarray(W, dtype=np.float32))
    b = np.asarray(b, dtype=np.float32)
    decay = np.asarray(decay, dtype=np.float32)
    thresh = np.asarray(thresh, dtype=np.float32)

    d = float(decay.reshape(-1)[0])
    th = float(thresh.reshape(-1)[0])
    has_bias = bool(np.any(b != 0))

    key = (MODE, d, th, has_bias)
    if key not in _CACHE:
        if MODE == "fp16x3":
            _CACHE[key] = build_kernel_fp16x3(d, th, has_bias)
        elif MODE == "hi16lo8":
            _CACHE[key] = build_kernel_hi16lo8(d, th, has_bias)
        else:
            _CACHE[key] = build_kernel_fp32hp(d, th, has_bias)
    nc = _CACHE[key]

    in_maps = []
    if MODE == "hi16lo8":
        import ml_dtypes

        FP8NP = ml_dtypes.float8_e4m3
        W16m = (W * np.float32(2.0**6)).astype(np.float16)
        W11 = W16m.astype(np.float32) * np.float32(2.0**-6)
        Wl = W - W11
        w16_l = _wt_layout(W16m)
        a1_l = _wt_layout((Wl * np.float32(2.0**17)).astype(FP8NP))
        b2_l = _wt_layout(W16m.astype(np.float32).astype(FP8NP))
        w8_l = np.ascontiguousarray(np.stack([a1_l, b2_l], axis=3))
        for i in range(N_CORES):
            xs_i = x[:, i * B_LOC : (i + 1) * B_LOC, :].reshape(TB, CI)
            X16m = (xs_i * np.float32(2.0**11)).astype(np.float16)
            x11 = X16m.astype(np.float32) * np.float32(2.0**-11)
            xl = xs_i - x11
            c1_l = _xt_layout(xs_i.astype(FP8NP))
            d2_l = _xt_layout((xl * np.float32(2.0**11)).astype(FP8NP))
            m = {
                "x16": _xt_layout(X16m),
                "x8": np.ascontiguousarray(np.stack([c1_l, d2_l], axis=2)),
                "w16": w16_l,
                "w8": w8_l,
            }
            if has_bias:
                m["bias"] = np.ascontiguousarray(b.reshape(CO, 1))
            in_maps.append(m)
    elif MODE == "fp16x3":
        Wh, Wl = _split16(W * np.float32(WSCALE))
        wh_l = _wt_layout(Wh)
        wl_l = _wt_layout(Wl)
        for i in range(N_CORES):
            xs_i = x[:, i * B_LOC : (i + 1) * B_LOC, :].reshape(TB, CI)
            xh_i, xl_i = _split16(xs_i * np.float32(XSCALE))
            m = {
                "xh": _xt_layout(xh_i),
                "xl": _xt_layout(xl_i),
                "wh": wh_l,
                "wl": wl_l,
            }
            if has_bias:
                m["bias"] = np.ascontiguousarray(
                    (b * np.float32(SCALE)).reshape(CO, 1)
                )
            in_maps.append(m)
    else:
        wt_l = _wt_layout(W)
        for i in range(N_CORES):
            xs_i = x[:, i * B_LOC : (i + 1) * B_LOC, :].reshape(TB, CI)
            m = {"xt": _xt_layout(xs_i), "wt": wt_l}
            if has_bias:
                m["bias"] = np.ascontiguousarray(b.reshape(CO, 1))
            in_maps.append(m)

    res = run_bass_kernel_spmd(
        nc, in_maps, core_ids=list(range(N_CORES)), trace=TRACE
    )
    LAST_EXEC_NS = res.exec_time_ns

    # spikes come back [CO, TB]; transpose to [T, B_loc, CO] per core
    out = np.concatenate(
        [
            np.ascontiguousarray(r["spkT"].T).reshape(T, B_LOC, CO)
            for r in res.results
        ],
        axis=1,
    )
    return np.ascontiguousarray(out)



# revision 9
# speedup vs baseline: 1.0235x; 1.0235x over previous
"""LIF spiking layer (T=32, B=256, C_in=C_out=4096, fp32) on 8 trn2 NeuronCores.

Strategy: data-parallel over batch (32 samples/core, W replicated).
Host-side numpy pre-permutes both operands into SBUF tile layout (contraction
dim ci on partitions), so each core only runs matmuls + the recurrence:
  current[co, (t,b)] = W @ x_core.T  on TensorE per 128-co tile (psum),
  LIF membrane recurrence over t on VectorE with mem laid out [co=128, b=32],
  spikes stored [co, (t,b)] and transposed back on the host.

MODE "fp32" is bit-exact vs the fp32 jax reference; "fp16x3" computes the
matmul as three fp16 hi/lo passes (25% faster, ~9e-4 rel err).
"""

import os

import numpy as np

import concourse.mybir as mybir
import concourse.tile as tile
from concourse import bacc
from concourse.bass_utils import run_bass_kernel_spmd

FP32 = mybir.dt.float32
FP16 = mybir.dt.float16

N_CORES = 8
T, B, CI, CO = 32, 256, 4096, 4096
B_LOC = B // N_CORES  # 32
TB = T * B_LOC  # 1024
# Exact powers of 2; the LIF recurrence is exactly scale-equivariant, and
# scaling keeps the fp16 lo-components out of subnormal range on the PE.
WSCALE = 64.0
XSCALE = 128.0
SCALE = WSCALE * XSCALE

# set by test.py to collect a profile
TRACE = False
LAST_EXEC_NS = None
# "hi16lo8": exact fp16 hi-pass + one fp8e4 DoubleRow correction pass
# (1.5 PE cycles/row, matmul rel err ~1e-5).
# "fp16x3": 3-pass fp16 hi/lo split matmul — rel err ~9e-4 (4 of 33.5M spikes
# flip), robust across ~70 device runs.
# "fp32": bit-exact vs the fp32 jax reference (0 mismatches) but native-fp32
# matmul streams intermittently wedge the exec unit on this hardware
# (NRT_EXEC_UNIT_UNRECOVERABLE in 2 of 5 runs), so it is not the default.
MODE = os.environ.get("LIF_KERNEL_MODE", "hi16lo8")

_CACHE = {}


def build_kernel_hi16lo8(
    d: float,
    th: float,
    has_bias: bool,
    T=T,
    B_loc=B_LOC,
    CI=CI,
    CO=CO,
):
    """2-pass matmul kernel: exact fp16 hi-pass + one fp8e4 DoubleRow pass
    that captures both cross terms via K-stacking (1.5 PE cycles/row total,
    matmul rel err ~1e-5):

      pass1: fp16(W*2^6).T @ fp16(x*2^11)                     (k=128/matmul)
      pass2: DoubleRow fp8 [Wl*2^17; W11*2^6] @ [x; xl*2^11]  (k=256/matmul)

    Both accumulate into one PSUM group at scale 2^17; the Activation-engine
    PSUM->SBUF drain applies 2^-17.

    Schedule: X pieces (96KB/partition fp16+fp8) stay resident in SBUF; W
    streams as 32 co-tiles (fp16+fp8, triple-buffered). co-tiles are
    processed in pairs ("groups"): PE accumulates both tiles' currents into
    PSUM, the Activation engine drains PSUM chunks into SBUF staging, and
    DVE runs the membrane recurrence for both tiles at once ([128, 64] per
    step), writing spikes in-place over the staged currents. Group g's
    recurrence overlaps group g+1's matmuls; spikes DMA out from the Pool
    engine (SWDGE) so the SP W-stream is never blocked. The first group
    consumes X k-tiles in DMA arrival order so the PE starts ~4us into the
    X stream.
    """
    FP8 = mybir.dt.float8e4
    TBl = T * B_loc
    n_k = CI // 128
    n_c = CO // 128
    csize = 512
    n_chunk = TBl // csize
    G = 2  # co-tiles per recurrence group
    n_g = n_c // G
    OSCALE = 2.0**-17

    nc = bacc.Bacc("TRN2", target_bir_lowering=False, debug=False, num_devices=N_CORES)

    x16 = nc.declare_dram_parameter("x16", [128, n_k, TBl], FP16, isOutput=False)
    x8 = nc.declare_dram_parameter("x8", [128, n_k, 2, TBl], FP8, isOutput=False)
    w16 = nc.declare_dram_parameter("w16", [n_c, 128, n_k, 128], FP16, isOutput=False)
    w8 = nc.declare_dram_parameter("w8", [n_c, 128, n_k, 2, 128], FP8, isOutput=False)
    if has_bias:
        bias = nc.declare_dram_parameter("bias", [CO, 1], FP32, isOutput=False)
    spkT = nc.declare_dram_parameter("spkT", [CO, TBl], FP32, isOutput=True)

    with tile.TileContext(nc) as tc:
        with (
            tc.tile_pool(name="xtp", bufs=1) as xt_pool,
            tc.tile_pool(name="wtp", bufs=3) as wt_pool,
            tc.tile_pool(name="stg", bufs=2) as stg_pool,
            tc.tile_pool(name="small", bufs=1) as small_pool,
            tc.tile_pool(name="pc", bufs=8, space="PSUM") as pc_pool,
        ):
            X16 = xt_pool.tile([128, n_k, TBl], FP16)
            X8 = xt_pool.tile([128, n_k, 2, TBl], FP8)
            mem = small_pool.tile([128, G * B_loc], FP32)
            mem1 = small_pool.tile([128, B_loc], FP32)

            W16s = [None] * n_c
            W8s = [None] * n_c

            def w16_tile(c, nm):
                W16s[c] = wt_pool.tile(
                    [128, n_k, 128], FP16, tag="w16", name=f"w16_{nm}"
                )

            def w8_tile(c, nm):
                W8s[c] = wt_pool.tile(
                    [128, n_k, 2, 128], FP8, tag="w8", name=f"w8_{nm}"
                )

            wq = 8  # k-slices per W strip
            def w16_strip(c, sidx):
                ks = slice(sidx * wq, (sidx + 1) * wq)
                nc.sync.dma_start(out=W16s[c][:, ks, :], in_=w16[c, :, ks, :])

            def w8_strip(c, sidx):
                ks = slice(sidx * wq, (sidx + 1) * wq)
                nc.sync.dma_start(out=W8s[c][:, ks, :, :], in_=w8[c, :, ks, :, :])

            # --- DMA program (SP queue), software-pipelined with the PE
            # phases below. Phase 1: X16 stream + W16 strips for tiles 0/1.
            for c in (0, 1, 2, 3):
                w16_tile(c, str(c))
                w8_tile(c, str(c))
            w16_strip(0, 0)
            w16_strip(1, 0)
            for k in range(n_k):
                nc.sync.dma_start(out=X16[:, k, :], in_=x16[:, k, :])
                if k in (5, 13, 21):
                    sidx = (k + 3) // 8
                    w16_strip(0, sidx)
                    w16_strip(1, sidx)
            # Phase 2: X8 stream + W8 strips for tiles 0/1 + W16 strips for
            # tiles 2/3 (their fp16 passes run inside phase 2 on the PE).
            for k in range(n_k):
                nc.sync.dma_start(out=X8[:, k, :, :], in_=x8[:, k, :, :])
                if k in (1, 9, 17, 25):
                    sidx = (k - 1) // 8
                    w8_strip(0, sidx)
                    w8_strip(1, sidx)
                    w16_strip(2, sidx)
                    w16_strip(3, sidx)
            # Phase 3 operands: tiles 2/3 fp8 pieces, then the steady W stream
            for c in (2, 3):
                nc.sync.dma_start(out=W8s[c], in_=w8[c, :, :, :, :])
            for c in range(4, n_c):
                w16_tile(c, "c")
                nc.sync.dma_start(out=W16s[c], in_=w16[c, :, :, :])
                w8_tile(c, "c")
                nc.sync.dma_start(out=W8s[c], in_=w8[c, :, :, :, :])
            if has_bias:
                b_tile = small_pool.tile([CO // 128, 128, 1], FP32)
                nc.sync.dma_start(
                    out=b_tile, in_=bias[:, :].reshape([CO // 128, 128, 1])
                )

            def mm16(pc, c, k, ch, start):
                nc.tensor.matmul(
                    pc,
                    lhsT=W16s[c][:, k, :],
                    rhs=X16[:, k, ch * csize : (ch + 1) * csize],
                    start=start,
                    stop=False,
                    skip_group_check=True,
                )

            def mm8(pc, c, k, ch, stop):
                nc.tensor.matmul(
                    pc,
                    lhsT=W8s[c][:, k, :, :],
                    rhs=X8[:, k, :, ch * csize : (ch + 1) * csize],
                    start=False,
                    stop=stop,
                    perf_mode=mybir.MatmulPerfMode.DoubleRow,
                    skip_group_check=True,
                )

            def drain(stage, pc, ti, ch):
                nc.scalar.activation(
                    stage[:, ti, ch * csize : (ch + 1) * csize],
                    pc,
                    mybir.ActivationFunctionType.Copy,
                    scale=OSCALE,
                )

            def recurrence(stage, c0, Gg, memt):
                """LIF recurrence for Gg co-tiles at once; spikes overwrite
                the staged currents; each finished chunk streams out via the
                Pool engine (SWDGE) so the SP W-stream is never blocked."""
                nc.vector.memset(memt, 0.0)
                for t in range(T):
                    o = t * B_loc
                    cur = stage[:, :, o : o + B_loc]
                    nc.vector.scalar_tensor_tensor(
                        out=memt,
                        in0=memt,
                        scalar=d,
                        in1=cur,
                        op0=mybir.AluOpType.mult,
                        op1=mybir.AluOpType.add,
                    )
                    if has_bias:
                        for ti in range(Gg):
                            nc.vector.tensor_scalar(
                                memt[:, ti * B_loc : (ti + 1) * B_loc],
                                memt[:, ti * B_loc : (ti + 1) * B_loc],
                                b_tile[c0 + ti],
                                None,
                                mybir.AluOpType.add,
                            )
                    nc.vector.tensor_scalar(
                        cur, memt, float(th), None, mybir.AluOpType.is_gt
                    )
                    nc.vector.scalar_tensor_tensor(
                        out=memt,
                        in0=cur,
                        scalar=-float(th),
                        in1=memt,
                        op0=mybir.AluOpType.mult,
                        op1=mybir.AluOpType.add,
                    )
                    if (t + 1) * B_loc % csize == 0:
                        cb = (t + 1) * B_loc - csize
                        for ti in range(Gg):
                            nc.gpsimd.dma_start(
                                out=spkT[
                                    (c0 + ti) * 128 : (c0 + ti + 1) * 128,
                                    cb : cb + csize,
                                ],
                                in_=stage[:, ti, cb : cb + csize],
                            )

            def pcs_alloc(Gg):
                return [
                    [
                        pc_pool.tile([128, csize], FP32, tag="pc", name="pc")
                        for _ in range(n_chunk)
                    ]
                    for _ in range(Gg)
                ]

            # --- tiles 0..3: pipelined prologue. Phase 1: fp16 passes of
            # tiles 0/1 k-outer, consuming X16 k-tiles in arrival order.
            stage_a = stg_pool.tile([128, G, TBl], FP32, tag="stage", name="stage_a")
            pcs_a = pcs_alloc(G)
            for k in range(n_k):
                for ti in range(G):
                    for ch in range(n_chunk):
                        mm16(pcs_a[ti][ch], ti, k, ch, k == 0)
            # Phase 2: DR passes of tiles 0/1 + fp16 passes of tiles 2/3,
            # consuming X8 k-tiles in arrival order.
            stage_b = stg_pool.tile([128, G, TBl], FP32, tag="stage", name="stage_b")
            pcs_b = pcs_alloc(G)
            for k in range(n_k):
                for ti in range(G):
                    for ch in range(n_chunk):
                        mm8(pcs_a[ti][ch], ti, k, ch, k == n_k - 1)
                for ti in range(G):
                    for ch in range(n_chunk):
                        mm16(pcs_b[ti][ch], 2 + ti, k, ch, k == 0)
            for ti in range(G):
                for ch in range(n_chunk):
                    drain(stage_a, pcs_a[ti][ch], ti, ch)
            # Phase 3: DR passes of tiles 2/3; then recurrence of tiles 0/1
            for ti in range(G):
                for ch in range(n_chunk):
                    for k in range(n_k):
                        mm8(pcs_b[ti][ch], 2 + ti, k, ch, k == n_k - 1)
                    drain(stage_b, pcs_b[ti][ch], ti, ch)
            recurrence(stage_a, 0, G, mem)
            recurrence(stage_b, 2, G, mem)

            # --- tiles 4..29: steady-state pair groups, ch-outer so the
            # recurrence's first half overlaps the group's second half.
            for g in range(2, n_g - 1):
                c0 = g * G
                stage = stg_pool.tile([128, G, TBl], FP32, tag="stage")
                pcs = pcs_alloc(G)
                for ch in range(n_chunk):
                    for ti in range(G):
                        for k in range(n_k):
                            mm16(pcs[ti][ch], c0 + ti, k, ch, k == 0)
                        for k in range(n_k):
                            mm8(pcs[ti][ch], c0 + ti, k, ch, k == n_k - 1)
                        drain(stage, pcs[ti][ch], ti, ch)
                recurrence(stage, c0, G, mem)

            # --- tiles 30/31: single-tile groups to shorten the final
            # recurrence tail (only the last tile's second half is exposed).
            for c in (n_c - 2, n_c - 1):
                stage = stg_pool.tile([128, 1, TBl], FP32, tag="stage1")
                pcs = pcs_alloc(1)
                for ch in range(n_chunk):
                    for k in range(n_k):
                        mm16(pcs[0][ch], c, k, ch, k == 0)
                    for k in range(n_k):
                        mm8(pcs[0][ch], c, k, ch, k == n_k - 1)
                    drain(stage, pcs[0][ch], 0, ch)
                recurrence(stage, c, 1, mem1)

    nc.compile()
    return nc


def build_kernel_fp16x3(
    d: float,
    th: float,
    has_bias: bool,
    T=T,
    B_loc=B_LOC,
    CI=CI,
    CO=CO,
):
    """3-pass fp16 hi/lo kernel. All operands arrive from the host already
    split, scaled, and permuted into SBUF tile layout, so the device does
    only matmuls + the recurrence. Spikes leave in [co, tb] layout."""
    TBl = T * B_loc
    n_k = CI // 128
    n_c = CO // 128
    csize = min(512, TBl)
    n_chunk = TBl // csize
    ths = float(th) * SCALE

    nc = bacc.Bacc("TRN2", target_bir_lowering=False, debug=False, num_devices=N_CORES)

    xh = nc.declare_dram_parameter("xh", [128, n_k, TBl], FP16, isOutput=False)
    xl = nc.declare_dram_parameter("xl", [128, n_k, TBl], FP16, isOutput=False)
    wh = nc.declare_dram_parameter("wh", [n_c, 128, n_k, 128], FP16, isOutput=False)
    wl = nc.declare_dram_parameter("wl", [n_c, 128, n_k, 128], FP16, isOutput=False)
    if has_bias:
        bias = nc.declare_dram_parameter("bias", [CO, 1], FP32, isOutput=False)
    spkT = nc.declare_dram_parameter("spkT", [CO, TBl], FP32, isOutput=True)

    with tile.TileContext(nc) as tc:
        with (
            tc.tile_pool(name="xt", bufs=1) as xt_pool,
            tc.tile_pool(name="wt", bufs=2) as wt_pool,
            tc.tile_pool(name="work", bufs=2) as work_pool,
            tc.tile_pool(name="pc", bufs=2 * n_chunk, space="PSUM") as pc_pool,
        ):
            XH = xt_pool.tile([128, n_k, TBl], FP16)
            XL = xt_pool.tile([128, n_k, TBl], FP16)
            # first W strips ahead of the X bulk on the same HWDGE FIFO
            WH_first = wt_pool.tile([128, n_k, 128], FP16, tag="wh")
            WL_first = wt_pool.tile([128, n_k, 128], FP16, tag="wl")
            wq = min(8, n_k)
            for kq in range(0, n_k, wq):
                nc.sync.dma_start(
                    out=WH_first[:, kq : kq + wq, :], in_=wh[0, :, kq : kq + wq, :]
                )
            nc.sync.dma_start(out=WL_first, in_=wl[0, :, :, :])
            for k in range(n_k):
                nc.sync.dma_start(out=XH[:, k, :], in_=xh[:, k, :])
                nc.sync.dma_start(out=XL[:, k, :], in_=xl[:, k, :])

            for c in range(n_c):
                if c == 0:
                    WH_c, WL_c = WH_first, WL_first
                else:
                    WH_c = wt_pool.tile([128, n_k, 128], FP16, tag="wh")
                    WL_c = wt_pool.tile([128, n_k, 128], FP16, tag="wl")
                    nc.sync.dma_start(out=WH_c, in_=wh[c, :, :, :])
                    nc.sync.dma_start(out=WL_c, in_=wl[c, :, :, :])
                if has_bias:
                    b_tile = work_pool.tile([128, 1], FP32, tag="bt")
                    nc.sync.dma_start(
                        out=b_tile, in_=bias[c * 128 : (c + 1) * 128, :]
                    )

                pcs = [
                    pc_pool.tile([128, csize], FP32, tag="pc", name="pc")
                    for _ in range(n_chunk)
                ]
                n_mm = 3 * n_k
                if c == 0:
                    # consume in DMA arrival order: all passes of k before k+1
                    order = [(k, p) for k in range(n_k) for p in (0, 1, 2)]
                else:
                    order = [(k, p) for p in (0, 1, 2) for k in range(n_k)]
                for ch in range(n_chunk):
                    ops = ((WH_c, XH), (WL_c, XH), (WH_c, XL))
                    for i, (k, p) in enumerate(order):
                        Wt, Xt = ops[p]
                        nc.tensor.matmul(
                            pcs[ch],
                            lhsT=Wt[:, k, :],
                            rhs=Xt[:, k, ch * csize : (ch + 1) * csize],
                            start=(i == 0),
                            stop=(i == n_mm - 1),
                        )

                mem = work_pool.tile([128, B_loc], FP32, tag="mem")
                s_stage = work_pool.tile([128, TBl], FP32, tag="s")
                nc.vector.memset(mem, 0.0)
                for t in range(T):
                    o = t * B_loc
                    cur = pcs[o // csize][:, o % csize : o % csize + B_loc]
                    nc.vector.scalar_tensor_tensor(
                        out=mem,
                        in0=mem,
                        scalar=d,
                        in1=cur,
                        op0=mybir.AluOpType.mult,
                        op1=mybir.AluOpType.add,
                    )
                    if has_bias:
                        nc.vector.tensor_scalar(
                            mem, mem, b_tile, None, mybir.AluOpType.add
                        )
                    s_t = s_stage[:, o : o + B_loc]
                    nc.vector.tensor_scalar(
                        s_t, mem, ths, None, mybir.AluOpType.is_gt
                    )
                    nc.vector.scalar_tensor_tensor(
                        out=mem,
                        in0=s_t,
                        scalar=-ths,
                        in1=mem,
                        op0=mybir.AluOpType.mult,
                        op1=mybir.AluOpType.add,
                    )

                nc.sync.dma_start(
                    out=spkT[c * 128 : (c + 1) * 128, :], in_=s_stage
                )

    nc.compile()
    return nc


def build_kernel_fp32hp(
    d: float,
    th: float,
    has_bias: bool,
    T=T,
    B_loc=B_LOC,
    CI=CI,
    CO=CO,
):
    """Exact-fp32 kernel with host-prepped transposed layouts: the device does
    only fp32 matmuls + the recurrence. Spikes leave in [co, tb] layout."""
    TBl = T * B_loc
    n_k = CI // 128
    n_c = CO // 128
    csize = min(512, TBl)
    n_chunk = TBl // csize

    nc = bacc.Bacc("TRN2", target_bir_lowering=False, debug=False, num_devices=N_CORES)

    xt = nc.declare_dram_parameter("xt", [128, n_k, TBl], FP32, isOutput=False)
    wt = nc.declare_dram_parameter("wt", [n_c, 128, n_k, 128], FP32, isOutput=False)
    if has_bias:
        bias = nc.declare_dram_parameter("bias", [CO, 1], FP32, isOutput=False)
    spkT = nc.declare_dram_parameter("spkT", [CO, TBl], FP32, isOutput=True)

    with tile.TileContext(nc) as tc:
        with (
            tc.tile_pool(name="xtp", bufs=1) as xt_pool,
            tc.tile_pool(name="wtp", bufs=3) as wt_pool,
            tc.tile_pool(name="work", bufs=2) as work_pool,
            tc.tile_pool(name="pc", bufs=4 * n_chunk, space="PSUM") as pc_pool,
        ):
            XT = xt_pool.tile([128, n_k, TBl], FP32)
            # first W strip ahead of the XT bulk on the same HWDGE FIFO, in
            # k-chunks, so co-tile 0's first matmuls start almost immediately
            WT_first = wt_pool.tile([128, n_k, 128], FP32, tag="wt")
            wq = min(8, n_k)
            for kq in range(0, n_k, wq):
                nc.sync.dma_start(
                    out=WT_first[:, kq : kq + wq, :], in_=wt[0, :, kq : kq + wq, :]
                )
            # per-k loads so co-tile 0 consumes tiles in DMA arrival order
            for k in range(n_k):
                nc.sync.dma_start(out=XT[:, k, :], in_=xt[:, k, :])

            for c in range(n_c):
                if c == 0:
                    WT_c = WT_first
                else:
                    WT_c = wt_pool.tile([128, n_k, 128], FP32, tag="wt")
                    nc.sync.dma_start(out=WT_c, in_=wt[c, :, :, :])
                if has_bias:
                    b_tile = work_pool.tile([128, 1], FP32, tag="bt")
                    nc.sync.dma_start(
                        out=b_tile, in_=bias[c * 128 : (c + 1) * 128, :]
                    )

                pcs = [
                    pc_pool.tile([128, csize], FP32, tag="pc", name="pc")
                    for _ in range(n_chunk)
                ]
                if c == 0:
                    # k outer: consume XT tiles as they arrive from DRAM
                    for k in range(n_k):
                        for ch in range(n_chunk):
                            nc.tensor.matmul(
                                pcs[ch],
                                lhsT=WT_c[:, k, :],
                                rhs=XT[:, k, ch * csize : (ch + 1) * csize],
                                start=(k == 0),
                                stop=(k == n_k - 1),
                            )
                else:
                    # chunk outer: chunk0 psum frees early for the recurrence
                    for ch in range(n_chunk):
                        for k in range(n_k):
                            nc.tensor.matmul(
                                pcs[ch],
                                lhsT=WT_c[:, k, :],
                                rhs=XT[:, k, ch * csize : (ch + 1) * csize],
                                start=(k == 0),
                                stop=(k == n_k - 1),
                            )

                mem = work_pool.tile([128, B_loc], FP32, tag="mem")
                s_stage = work_pool.tile([128, TBl], FP32, tag="s")
                nc.vector.memset(mem, 0.0)
                for t in range(T):
                    o = t * B_loc
                    cur = pcs[o // csize][:, o % csize : o % csize + B_loc]
                    nc.vector.scalar_tensor_tensor(
                        out=mem,
                        in0=mem,
                        scalar=d,
                        in1=cur,
                        op0=mybir.AluOpType.mult,
                        op1=mybir.AluOpType.add,
                    )
                    if has_bias:
                        nc.vector.tensor_scalar(
                            mem, mem, b_tile, None, mybir.AluOpType.add
                        )
                    s_t = s_stage[:, o : o + B_loc]
                    nc.vector.tensor_scalar(
                        s_t, mem, float(th), None, mybir.AluOpType.is_gt
                    )
                    nc.vector.scalar_tensor_tensor(
                        out=mem,
                        in0=s_t,
                        scalar=-float(th),
                        in1=mem,
                        op0=mybir.AluOpType.mult,
                        op1=mybir.AluOpType.add,
                    )

                nc.sync.dma_start(
                    out=spkT[c * 128 : (c + 1) * 128, :], in_=s_stage
                )

    nc.compile()
    return nc


def _split16(a32):
    hi = a32.astype(np.float16)
    lo = (a32 - hi.astype(np.float32)).astype(np.float16)
    return hi, lo


def _xt_layout(xs):
    """[TB, CI] -> [128, CI//128, TB] so SBUF partition p holds ci = k*128+p."""
    TBl, CIl = xs.shape
    return np.ascontiguousarray(
        xs.reshape(TBl, CIl // 128, 128).transpose(2, 1, 0)
    )


def _wt_layout(Wm):
    """[CO, CI] -> [CO//128, 128, CI//128, 128]: strip c, partition p=ci%128,
    k=ci//128, j=co%128 -> W[c*128+j, k*128+p]."""
    COl, CIl = Wm.shape
    return np.ascontiguousarray(
        Wm.reshape(COl // 128, 128, CIl // 128, 128).transpose(0, 3, 2, 1)
    )


def kernel(x, W, b, decay, thresh):
    global LAST_EXEC_NS
    x = np.ascontiguousarray(np.asarray(x, dtype=np.float32))
    W = np.ascontiguousarray(np.asarray(W, dtype=np.float32))
    b = np.asarray(b, dtype=np.float32)
    decay = np.asarray(decay, dtype=np.float32)
    thresh = np.asarray(thresh, dtype=np.float32)

    d = float(decay.reshape(-1)[0])
    th = float(thresh.reshape(-1)[0])
    has_bias = bool(np.any(b != 0))

    key = (MODE, d, th, has_bias)
    if key not in _CACHE:
        if MODE == "fp16x3":
            _CACHE[key] = build_kernel_fp16x3(d, th, has_bias)
        elif MODE == "hi16lo8":
            _CACHE[key] = build_kernel_hi16lo8(d, th, has_bias)
        else:
            _CACHE[key] = build_kernel_fp32hp(d, th, has_bias)
    nc = _CACHE[key]

    in_maps = []
    if MODE == "hi16lo8":
        import ml_dtypes

        FP8NP = ml_dtypes.float8_e4m3
        W16m = (W * np.float32(2.0**6)).astype(np.float16)
        W11 = W16m.astype(np.float32) * np.float32(2.0**-6)
        Wl = W - W11
        w16_l = _wt_layout(W16m)
        a1_l = _wt_layout((Wl * np.float32(2.0**17)).astype(FP8NP))
        b2_l = _wt_layout(W16m.astype(np.float32).astype(FP8NP))
        w8_l = np.ascontiguousarray(np.stack([a1_l, b2_l], axis=3))
        for i in range(N_CORES):
            xs_i = x[:, i * B_LOC : (i + 1) * B_LOC, :].reshape(TB, CI)
            X16m = (xs_i * np.float32(2.0**11)).astype(np.float16)
            x11 = X16m.astype(np.float32) * np.float32(2.0**-11)
            xl = xs_i - x11
            c1_l = _xt_layout(xs_i.astype(FP8NP))
            d2_l = _xt_layout((xl * np.float32(2.0**11)).astype(FP8NP))
            m = {
                "x16": _xt_layout(X16m),
                "x8": np.ascontiguousarray(np.stack([c1_l, d2_l], axis=2)),
                "w16": w16_l,
                "w8": w8_l,
            }
            if has_bias:
                m["bias"] = np.ascontiguousarray(b.reshape(CO, 1))
            in_maps.append(m)
    elif MODE == "fp16x3":
        Wh, Wl = _split16(W * np.float32(WSCALE))
        wh_l = _wt_layout(Wh)
        wl_l = _wt_layout(Wl)
        for i in range(N_CORES):
            xs_i = x[:, i * B_LOC : (i + 1) * B_LOC, :].reshape(TB, CI)
            xh_i, xl_i = _split16(xs_i * np.float32(XSCALE))
            m = {
                "xh": _xt_layout(xh_i),
                "xl": _xt_layout(xl_i),
                "wh": wh_l,
                "wl": wl_l,
            }
            if has_bias:
                m["bias"] = np.ascontiguousarray(
                    (b * np.float32(SCALE)).reshape(CO, 1)
                )
            in_maps.append(m)
    else:
        wt_l = _wt_layout(W)
        for i in range(N_CORES):
            xs_i = x[:, i * B_LOC : (i + 1) * B_LOC, :].reshape(TB, CI)
            m = {"xt": _xt_layout(xs_i), "wt": wt_l}
            if has_bias:
                m["bias"] = np.ascontiguousarray(b.reshape(CO, 1))
            in_maps.append(m)

    res = run_bass_kernel_spmd(
        nc, in_maps, core_ids=list(range(N_CORES)), trace=TRACE
    )
    LAST_EXEC_NS = res.exec_time_ns

    # spikes come back [CO, TB]; transpose to [T, B_loc, CO] per core
    out = np.concatenate(
        [
            np.ascontiguousarray(r["spkT"].T).reshape(T, B_LOC, CO)
            for r in res.results
        ],
        axis=1,
    )
    return np.ascontiguousarray(out)



# revision 10
# speedup vs baseline: 1.0290x; 1.0054x over previous
"""LIF spiking layer (T=32, B=256, C_in=C_out=4096, fp32) on 8 trn2 NeuronCores.

Strategy: data-parallel over batch (32 samples/core, W replicated).
Host-side numpy pre-permutes both operands into SBUF tile layout (contraction
dim ci on partitions), so each core only runs matmuls + the recurrence:
  current[co, (t,b)] = W @ x_core.T  on TensorE per 128-co tile (psum),
  LIF membrane recurrence over t on VectorE with mem laid out [co=128, b=32],
  spikes stored [co, (t,b)] and transposed back on the host.

MODE "fp32" is bit-exact vs the fp32 jax reference; "fp16x3" computes the
matmul as three fp16 hi/lo passes (25% faster, ~9e-4 rel err).
"""

import os

import numpy as np

import concourse.mybir as mybir
import concourse.tile as tile
from concourse import bacc
from concourse.bass_utils import run_bass_kernel_spmd

FP32 = mybir.dt.float32
FP16 = mybir.dt.float16

N_CORES = 8
T, B, CI, CO = 32, 256, 4096, 4096
B_LOC = B // N_CORES  # 32
TB = T * B_LOC  # 1024
# Exact powers of 2; the LIF recurrence is exactly scale-equivariant, and
# scaling keeps the fp16 lo-components out of subnormal range on the PE.
WSCALE = 64.0
XSCALE = 128.0
SCALE = WSCALE * XSCALE

# set by test.py to collect a profile
TRACE = False
LAST_EXEC_NS = None
# "hi16lo8": exact fp16 hi-pass + one fp8e4 DoubleRow correction pass
# (1.5 PE cycles/row, matmul rel err ~1e-5).
# "fp16x3": 3-pass fp16 hi/lo split matmul — rel err ~9e-4 (4 of 33.5M spikes
# flip), robust across ~70 device runs.
# "fp32": bit-exact vs the fp32 jax reference (0 mismatches) but native-fp32
# matmul streams intermittently wedge the exec unit on this hardware
# (NRT_EXEC_UNIT_UNRECOVERABLE in 2 of 5 runs), so it is not the default.
MODE = os.environ.get("LIF_KERNEL_MODE", "hi16lo8")

_CACHE = {}


def build_kernel_hi16lo8(
    d: float,
    th: float,
    has_bias: bool,
    T=T,
    B_loc=B_LOC,
    CI=CI,
    CO=CO,
):
    """2-pass matmul kernel: exact fp16 hi-pass + one fp8e4 DoubleRow pass
    that captures both cross terms via K-stacking (1.5 PE cycles/row total,
    matmul rel err ~1e-5):

      pass1: fp16(W*2^6).T @ fp16(x*2^11)                     (k=128/matmul)
      pass2: DoubleRow fp8 [Wl*2^17; W11*2^6] @ [x; xl*2^11]  (k=256/matmul)

    Both accumulate into one PSUM group at scale 2^17; the Activation-engine
    PSUM->SBUF drain applies 2^-17.

    Schedule: X pieces (96KB/partition fp16+fp8) stay resident in SBUF; W
    streams as 32 co-tiles (fp16+fp8, triple-buffered). co-tiles are
    processed in pairs ("groups"): PE accumulates both tiles' currents into
    PSUM, the Activation engine drains PSUM chunks into SBUF staging, and
    DVE runs the membrane recurrence for both tiles at once ([128, 64] per
    step), writing spikes in-place over the staged currents. Group g's
    recurrence overlaps group g+1's matmuls; spikes DMA out from the Pool
    engine (SWDGE) so the SP W-stream is never blocked. The first group
    consumes X k-tiles in DMA arrival order so the PE starts ~4us into the
    X stream.
    """
    FP8 = mybir.dt.float8e4
    TBl = T * B_loc
    n_k = CI // 128
    n_c = CO // 128
    csize = 512
    n_chunk = TBl // csize
    G = 2  # co-tiles per recurrence group
    n_g = n_c // G
    OSCALE = 2.0**-17

    nc = bacc.Bacc("TRN2", target_bir_lowering=False, debug=False, num_devices=N_CORES)

    x16 = nc.declare_dram_parameter("x16", [128, n_k, TBl], FP16, isOutput=False)
    x8 = nc.declare_dram_parameter("x8", [128, n_k, 2, TBl], FP8, isOutput=False)
    w16 = nc.declare_dram_parameter("w16", [n_c, 128, n_k, 128], FP16, isOutput=False)
    w8 = nc.declare_dram_parameter("w8", [n_c, 128, n_k, 2, 128], FP8, isOutput=False)
    if has_bias:
        bias = nc.declare_dram_parameter("bias", [CO, 1], FP32, isOutput=False)
    spkT = nc.declare_dram_parameter("spkT", [CO, TBl], FP32, isOutput=True)

    with tile.TileContext(nc) as tc:
        with (
            tc.tile_pool(name="xtp", bufs=1) as xt_pool,
            tc.tile_pool(name="wtp", bufs=3) as wt_pool,
            tc.tile_pool(name="stg", bufs=2) as stg_pool,
            tc.tile_pool(name="small", bufs=1) as small_pool,
            tc.tile_pool(name="pc", bufs=8, space="PSUM") as pc_pool,
        ):
            X16 = xt_pool.tile([128, n_k, TBl], FP16)
            X8 = xt_pool.tile([128, n_k, 2, TBl], FP8)
            mem = small_pool.tile([128, G * B_loc], FP32)
            mem1 = small_pool.tile([128, B_loc], FP32)

            W16s = [None] * n_c
            W8s = [None] * n_c

            def w16_tile(c, nm):
                W16s[c] = wt_pool.tile(
                    [128, n_k, 128], FP16, tag="w16", name=f"w16_{nm}"
                )

            def w8_tile(c, nm):
                W8s[c] = wt_pool.tile(
                    [128, n_k, 2, 128], FP8, tag="w8", name=f"w8_{nm}"
                )

            wq = 8  # k-slices per W strip
            def w16_strip(c, sidx):
                ks = slice(sidx * wq, (sidx + 1) * wq)
                nc.sync.dma_start(out=W16s[c][:, ks, :], in_=w16[c, :, ks, :])

            def w8_strip(c, sidx):
                ks = slice(sidx * wq, (sidx + 1) * wq)
                nc.sync.dma_start(out=W8s[c][:, ks, :, :], in_=w8[c, :, ks, :, :])

            # --- DMA program (SP queue), software-pipelined with the PE
            # phases below. Phase 1: X16 stream + W16 strips for tiles 0/1.
            for c in (0, 1, 2, 3):
                w16_tile(c, str(c))
                w8_tile(c, str(c))
            w16_strip(0, 0)
            for k in range(n_k):
                nc.sync.dma_start(out=X16[:, k, :], in_=x16[:, k, :])
                if k == 0:
                    w16_strip(1, 0)
                if k in (5, 13, 21):
                    sidx = (k + 3) // 8
                    w16_strip(0, sidx)
                    w16_strip(1, sidx)
            # Phase 2: X8 stream + W8 strips for tiles 0/1 + W16 strips for
            # tiles 2/3 (their fp16 passes run inside phase 2 on the PE).
            for k in range(n_k):
                nc.sync.dma_start(out=X8[:, k, :, :], in_=x8[:, k, :, :])
                if k in (1, 9, 17, 25):
                    sidx = (k - 1) // 8
                    w8_strip(0, sidx)
                    w8_strip(1, sidx)
                    w16_strip(2, sidx)
                    w16_strip(3, sidx)
            # Phase 3 operands: tiles 2/3 fp8 pieces, then the steady W stream
            for c in (2, 3):
                nc.sync.dma_start(out=W8s[c], in_=w8[c, :, :, :, :])
            for c in range(4, n_c):
                w16_tile(c, "c")
                nc.sync.dma_start(out=W16s[c], in_=w16[c, :, :, :])
                w8_tile(c, "c")
                nc.sync.dma_start(out=W8s[c], in_=w8[c, :, :, :, :])
            if has_bias:
                b_tile = small_pool.tile([CO // 128, 128, 1], FP32)
                nc.sync.dma_start(
                    out=b_tile, in_=bias[:, :].reshape([CO // 128, 128, 1])
                )

            def mm16(pc, c, k, ch, start):
                nc.tensor.matmul(
                    pc,
                    lhsT=W16s[c][:, k, :],
                    rhs=X16[:, k, ch * csize : (ch + 1) * csize],
                    start=start,
                    stop=False,
                    skip_group_check=True,
                )

            def mm8(pc, c, k, ch, stop):
                nc.tensor.matmul(
                    pc,
                    lhsT=W8s[c][:, k, :, :],
                    rhs=X8[:, k, :, ch * csize : (ch + 1) * csize],
                    start=False,
                    stop=stop,
                    perf_mode=mybir.MatmulPerfMode.DoubleRow,
                    skip_group_check=True,
                )

            def drain(stage, pc, ti, ch):
                nc.scalar.activation(
                    stage[:, ti, ch * csize : (ch + 1) * csize],
                    pc,
                    mybir.ActivationFunctionType.Copy,
                    scale=OSCALE,
                )

            def drain_w(stage, pc_ap, ti, col0, width):
                nc.scalar.activation(
                    stage[:, ti, col0 : col0 + width],
                    pc_ap,
                    mybir.ActivationFunctionType.Copy,
                    scale=OSCALE,
                )

            def recurrence(stage, c0, Gg, memt, ostep=csize):
                """LIF recurrence for Gg co-tiles at once; spikes overwrite
                the staged currents; each finished chunk streams out via the
                Pool engine (SWDGE) so the SP W-stream is never blocked."""
                nc.vector.memset(memt, 0.0)
                for t in range(T):
                    o = t * B_loc
                    cur = stage[:, :, o : o + B_loc]
                    nc.vector.scalar_tensor_tensor(
                        out=memt,
                        in0=memt,
                        scalar=d,
                        in1=cur,
                        op0=mybir.AluOpType.mult,
                        op1=mybir.AluOpType.add,
                    )
                    if has_bias:
                        for ti in range(Gg):
                            nc.vector.tensor_scalar(
                                memt[:, ti * B_loc : (ti + 1) * B_loc],
                                memt[:, ti * B_loc : (ti + 1) * B_loc],
                                b_tile[c0 + ti],
                                None,
                                mybir.AluOpType.add,
                            )
                    nc.vector.tensor_scalar(
                        cur, memt, float(th), None, mybir.AluOpType.is_gt
                    )
                    nc.vector.scalar_tensor_tensor(
                        out=memt,
                        in0=cur,
                        scalar=-float(th),
                        in1=memt,
                        op0=mybir.AluOpType.mult,
                        op1=mybir.AluOpType.add,
                    )
                    if (t + 1) * B_loc % ostep == 0:
                        cb = (t + 1) * B_loc - ostep
                        for ti in range(Gg):
                            nc.gpsimd.dma_start(
                                out=spkT[
                                    (c0 + ti) * 128 : (c0 + ti + 1) * 128,
                                    cb : cb + ostep,
                                ],
                                in_=stage[:, ti, cb : cb + ostep],
                            )

            def pcs_alloc(Gg):
                return [
                    [
                        pc_pool.tile([128, csize], FP32, tag="pc", name="pc")
                        for _ in range(n_chunk)
                    ]
                    for _ in range(Gg)
                ]

            # --- tiles 0..3: pipelined prologue. Phase 1: fp16 passes of
            # tiles 0/1 k-outer, consuming X16 k-tiles in arrival order.
            stage_a = stg_pool.tile([128, G, TBl], FP32, tag="stage", name="stage_a")
            pcs_a = pcs_alloc(G)
            for k in range(n_k):
                for ti in range(G):
                    for ch in range(n_chunk):
                        mm16(pcs_a[ti][ch], ti, k, ch, k == 0)
            # Phase 2: DR passes of tiles 0/1 + fp16 passes of tiles 2/3,
            # consuming X8 k-tiles in arrival order.
            stage_b = stg_pool.tile([128, G, TBl], FP32, tag="stage", name="stage_b")
            pcs_b = pcs_alloc(G)
            for k in range(n_k):
                for ti in range(G):
                    for ch in range(n_chunk):
                        mm8(pcs_a[ti][ch], ti, k, ch, k == n_k - 1)
                for ti in range(G):
                    for ch in range(n_chunk):
                        mm16(pcs_b[ti][ch], 2 + ti, k, ch, k == 0)
            for ti in range(G):
                for ch in range(n_chunk):
                    drain(stage_a, pcs_a[ti][ch], ti, ch)
            # Phase 3: DR passes of tiles 2/3; then recurrence of tiles 0/1
            for ti in range(G):
                for ch in range(n_chunk):
                    for k in range(n_k):
                        mm8(pcs_b[ti][ch], 2 + ti, k, ch, k == n_k - 1)
                    drain(stage_b, pcs_b[ti][ch], ti, ch)
            recurrence(stage_a, 0, G, mem)
            recurrence(stage_b, 2, G, mem)

            # --- tiles 4..29: steady-state pair groups, ch-outer so the
            # recurrence's first half overlaps the group's second half.
            for g in range(2, n_g - 1):
                c0 = g * G
                stage = stg_pool.tile([128, G, TBl], FP32, tag="stage")
                pcs = pcs_alloc(G)
                for ch in range(n_chunk):
                    for ti in range(G):
                        for k in range(n_k):
                            mm16(pcs[ti][ch], c0 + ti, k, ch, k == 0)
                        for k in range(n_k):
                            mm8(pcs[ti][ch], c0 + ti, k, ch, k == n_k - 1)
                        drain(stage, pcs[ti][ch], ti, ch)
                recurrence(stage, c0, G, mem)

            # --- tiles 30/31: single-tile groups to shorten the final
            # recurrence tail; the very last tile uses 256-col sub-chunks so
            # only its final quarter's recurrence is exposed past PE end.
            c = n_c - 2
            stage30 = stg_pool.tile([128, 1, TBl], FP32, tag="stage1")
            pcs = pcs_alloc(1)
            for ch in range(n_chunk):
                for k in range(n_k):
                    mm16(pcs[0][ch], c, k, ch, k == 0)
                for k in range(n_k):
                    mm8(pcs[0][ch], c, k, ch, k == n_k - 1)
                drain(stage30, pcs[0][ch], 0, ch)
            recurrence(stage30, c, 1, mem1)

            c = n_c - 1
            qs = 256
            stage31 = stg_pool.tile([128, 1, TBl], FP32, tag="stage1")
            pcs = pcs_alloc(1)
            for q in range(TBl // qs):
                pc_ap = pcs[0][q // 2][:, (q % 2) * qs : (q % 2 + 1) * qs]
                col0 = q * qs
                for k in range(n_k):
                    nc.tensor.matmul(
                        pc_ap,
                        lhsT=W16s[c][:, k, :],
                        rhs=X16[:, k, col0 : col0 + qs],
                        start=(k == 0),
                        stop=False,
                        skip_group_check=True,
                    )
                for k in range(n_k):
                    nc.tensor.matmul(
                        pc_ap,
                        lhsT=W8s[c][:, k, :, :],
                        rhs=X8[:, k, :, col0 : col0 + qs],
                        start=False,
                        stop=(k == n_k - 1),
                        perf_mode=mybir.MatmulPerfMode.DoubleRow,
                        skip_group_check=True,
                    )
                drain_w(stage31, pc_ap, 0, col0, qs)
            recurrence(stage31, c, 1, mem1, ostep=qs)

    nc.compile()
    return nc


def build_kernel_fp16x3(
    d: float,
    th: float,
    has_bias: bool,
    T=T,
    B_loc=B_LOC,
    CI=CI,
    CO=CO,
):
    """3-pass fp16 hi/lo kernel. All operands arrive from the host already
    split, scaled, and permuted into SBUF tile layout, so the device does
    only matmuls + the recurrence. Spikes leave in [co, tb] layout."""
    TBl = T * B_loc
    n_k = CI // 128
    n_c = CO // 128
    csize = min(512, TBl)
    n_chunk = TBl // csize
    ths = float(th) * SCALE

    nc = bacc.Bacc("TRN2", target_bir_lowering=False, debug=False, num_devices=N_CORES)

    xh = nc.declare_dram_parameter("xh", [128, n_k, TBl], FP16, isOutput=False)
    xl = nc.declare_dram_parameter("xl", [128, n_k, TBl], FP16, isOutput=False)
    wh = nc.declare_dram_parameter("wh", [n_c, 128, n_k, 128], FP16, isOutput=False)
    wl = nc.declare_dram_parameter("wl", [n_c, 128, n_k, 128], FP16, isOutput=False)
    if has_bias:
        bias = nc.declare_dram_parameter("bias", [CO, 1], FP32, isOutput=False)
    spkT = nc.declare_dram_parameter("spkT", [CO, TBl], FP32, isOutput=True)

    with tile.TileContext(nc) as tc:
        with (
            tc.tile_pool(name="xt", bufs=1) as xt_pool,
            tc.tile_pool(name="wt", bufs=2) as wt_pool,
            tc.tile_pool(name="work", bufs=2) as work_pool,
            tc.tile_pool(name="pc", bufs=2 * n_chunk, space="PSUM") as pc_pool,
        ):
            XH = xt_pool.tile([128, n_k, TBl], FP16)
            XL = xt_pool.tile([128, n_k, TBl], FP16)
            # first W strips ahead of the X bulk on the same HWDGE FIFO
            WH_first = wt_pool.tile([128, n_k, 128], FP16, tag="wh")
            WL_first = wt_pool.tile([128, n_k, 128], FP16, tag="wl")
            wq = min(8, n_k)
            for kq in range(0, n_k, wq):
                nc.sync.dma_start(
                    out=WH_first[:, kq : kq + wq, :], in_=wh[0, :, kq : kq + wq, :]
                )
            nc.sync.dma_start(out=WL_first, in_=wl[0, :, :, :])
            for k in range(n_k):
                nc.sync.dma_start(out=XH[:, k, :], in_=xh[:, k, :])
                nc.sync.dma_start(out=XL[:, k, :], in_=xl[:, k, :])

            for c in range(n_c):
                if c == 0:
                    WH_c, WL_c = WH_first, WL_first
                else:
                    WH_c = wt_pool.tile([128, n_k, 128], FP16, tag="wh")
                    WL_c = wt_pool.tile([128, n_k, 128], FP16, tag="wl")
                    nc.sync.dma_start(out=WH_c, in_=wh[c, :, :, :])
                    nc.sync.dma_start(out=WL_c, in_=wl[c, :, :, :])
                if has_bias:
                    b_tile = work_pool.tile([128, 1], FP32, tag="bt")
                    nc.sync.dma_start(
                        out=b_tile, in_=bias[c * 128 : (c + 1) * 128, :]
                    )

                pcs = [
                    pc_pool.tile([128, csize], FP32, tag="pc", name="pc")
                    for _ in range(n_chunk)
                ]
                n_mm = 3 * n_k
                if c == 0:
                    # consume in DMA arrival order: all passes of k before k+1
                    order = [(k, p) for k in range(n_k) for p in (0, 1, 2)]
                else:
                    order = [(k, p) for p in (0, 1, 2) for k in range(n_k)]
                for ch in range(n_chunk):
                    ops = ((WH_c, XH), (WL_c, XH), (WH_c, XL))
                    for i, (k, p) in enumerate(order):
                        Wt, Xt = ops[p]
                        nc.tensor.matmul(
                            pcs[ch],
                            lhsT=Wt[:, k, :],
                            rhs=Xt[:, k, ch * csize : (ch + 1) * csize],
                            start=(i == 0),
                            stop=(i == n_mm - 1),
                        )

                mem = work_pool.tile([128, B_loc], FP32, tag="mem")
                s_stage = work_pool.tile([128, TBl], FP32, tag="s")
                nc.vector.memset(mem, 0.0)
                for t in range(T):
                    o = t * B_loc
                    cur = pcs[o // csize][:, o % csize : o % csize + B_loc]
                    nc.vector.scalar_tensor_tensor(
                        out=mem,
                        in0=mem,
                        scalar=d,
                        in1=cur,
                        op0=mybir.AluOpType.mult,
                        op1=mybir.AluOpType.add,
                    )
                    if has_bias:
                        nc.vector.tensor_scalar(
                            mem, mem, b_tile, None, mybir.AluOpType.add
                        )
                    s_t = s_stage[:, o : o + B_loc]
                    nc.vector.tensor_scalar(
                        s_t, mem, ths, None, mybir.AluOpType.is_gt
                    )
                    nc.vector.scalar_tensor_tensor(
                        out=mem,
                        in0=s_t,
                        scalar=-ths,
                        in1=mem,
                        op0=mybir.AluOpType.mult,
                        op1=mybir.AluOpType.add,
                    )

                nc.sync.dma_start(
                    out=spkT[c * 128 : (c + 1) * 128, :], in_=s_stage
                )

    nc.compile()
    return nc


def build_kernel_fp32hp(
    d: float,
    th: float,
    has_bias: bool,
    T=T,
    B_loc=B_LOC,
    CI=CI,
    CO=CO,
):
    """Exact-fp32 kernel with host-prepped transposed layouts: the device does
    only fp32 matmuls + the recurrence. Spikes leave in [co, tb] layout."""
    TBl = T * B_loc
    n_k = CI // 128
    n_c = CO // 128
    csize = min(512, TBl)
    n_chunk = TBl // csize

    nc = bacc.Bacc("TRN2", target_bir_lowering=False, debug=False, num_devices=N_CORES)

    xt = nc.declare_dram_parameter("xt", [128, n_k, TBl], FP32, isOutput=False)
    wt = nc.declare_dram_parameter("wt", [n_c, 128, n_k, 128], FP32, isOutput=False)
    if has_bias:
        bias = nc.declare_dram_parameter("bias", [CO, 1], FP32, isOutput=False)
    spkT = nc.declare_dram_parameter("spkT", [CO, TBl], FP32, isOutput=True)

    with tile.TileContext(nc) as tc:
        with (
            tc.tile_pool(name="xtp", bufs=1) as xt_pool,
            tc.tile_pool(name="wtp", bufs=3) as wt_pool,
            tc.tile_pool(name="work", bufs=2) as work_pool,
            tc.tile_pool(name="pc", bufs=4 * n_chunk, space="PSUM") as pc_pool,
        ):
            XT = xt_pool.tile([128, n_k, TBl], FP32)
            # first W strip ahead of the XT bulk on the same HWDGE FIFO, in
            # k-chunks, so co-tile 0's first matmuls start almost immediately
            WT_first = wt_pool.tile([128, n_k, 128], FP32, tag="wt")
            wq = min(8, n_k)
            for kq in range(0, n_k, wq):
                nc.sync.dma_start(
                    out=WT_first[:, kq : kq + wq, :], in_=wt[0, :, kq : kq + wq, :]
                )
            # per-k loads so co-tile 0 consumes tiles in DMA arrival order
            for k in range(n_k):
                nc.sync.dma_start(out=XT[:, k, :], in_=xt[:, k, :])

            for c in range(n_c):
                if c == 0:
                    WT_c = WT_first
                else:
                    WT_c = wt_pool.tile([128, n_k, 128], FP32, tag="wt")
                    nc.sync.dma_start(out=WT_c, in_=wt[c, :, :, :])
                if has_bias:
                    b_tile = work_pool.tile([128, 1], FP32, tag="bt")
                    nc.sync.dma_start(
                        out=b_tile, in_=bias[c * 128 : (c + 1) * 128, :]
                    )

                pcs = [
                    pc_pool.tile([128, csize], FP32, tag="pc", name="pc")
                    for _ in range(n_chunk)
                ]
                if c == 0:
                    # k outer: consume XT tiles as they arrive from DRAM
                    for k in range(n_k):
                        for ch in range(n_chunk):
                            nc.tensor.matmul(
                                pcs[ch],
                                lhsT=WT_c[:, k, :],
                                rhs=XT[:, k, ch * csize : (ch + 1) * csize],
                                start=(k == 0),
                                stop=(k == n_k - 1),
                            )
                else:
                    # chunk outer: chunk0 psum frees early for the recurrence
                    for ch in range(n_chunk):
                        for k in range(n_k):
                            nc.tensor.matmul(
                                pcs[ch],
                                lhsT=WT_c[:, k, :],
                                rhs=XT[:, k, ch * csize : (ch + 1) * csize],
                                start=(k == 0),
                                stop=(k == n_k - 1),
                            )

                mem = work_pool.tile([128, B_loc], FP32, tag="mem")
                s_stage = work_pool.tile([128, TBl], FP32, tag="s")
                nc.vector.memset(mem, 0.0)
                for t in range(T):
                    o = t * B_loc
                    cur = pcs[o // csize][:, o % csize : o % csize + B_loc]
                    nc.vector.scalar_tensor_tensor(
                        out=mem,
                        in0=mem,
                        scalar=d,
                        in1=cur,
                        op0=mybir.AluOpType.mult,
                        op1=mybir.AluOpType.add,
                    )
                    if has_bias:
                        nc.vector.tensor_scalar(
                            mem, mem, b_tile, None, mybir.AluOpType.add
                        )
                    s_t = s_stage[:, o : o + B_loc]
                    nc.vector.tensor_scalar(
                        s_t, mem, float(th), None, mybir.AluOpType.is_gt
                    )
                    nc.vector.scalar_tensor_tensor(
                        out=mem,
                        in0=s_t,
                        scalar=-float(th),
                        in1=mem,
                        op0=mybir.AluOpType.mult,
                        op1=mybir.AluOpType.add,
                    )

                nc.sync.dma_start(
                    out=spkT[c * 128 : (c + 1) * 128, :], in_=s_stage
                )

    nc.compile()
    return nc


def _split16(a32):
    hi = a32.astype(np.float16)
    lo = (a32 - hi.astype(np.float32)).astype(np.float16)
    return hi, lo


def _xt_layout(xs):
    """[TB, CI] -> [128, CI//128, TB] so SBUF partition p holds ci = k*128+p."""
    TBl, CIl = xs.shape
    return np.ascontiguousarray(
        xs.reshape(TBl, CIl // 128, 128).transpose(2, 1, 0)
    )


def _wt_layout(Wm):
    """[CO, CI] -> [CO//128, 128, CI//128, 128]: strip c, partition p=ci%128,
    k=ci//128, j=co%128 -> W[c*128+j, k*128+p]."""
    COl, CIl = Wm.shape
    return np.ascontiguousarray(
        Wm.reshape(COl // 128, 128, CIl // 128, 128).transpose(0, 3, 2, 1)
    )


def kernel(x, W, b, decay, thresh):
    global LAST_EXEC_NS
    x = np.ascontiguousarray(np.asarray(x, dtype=np.float32))
    W = np.ascontiguousarray(np.asarray(W, dtype=np.float32))
    b = np.asarray(b, dtype=np.float32)
    decay = np.asarray(decay, dtype=np.float32)
    thresh = np.asarray(thresh, dtype=np.float32)

    d = float(decay.reshape(-1)[0])
    th = float(thresh.reshape(-1)[0])
    has_bias = bool(np.any(b != 0))

    key = (MODE, d, th, has_bias)
    if key not in _CACHE:
        if MODE == "fp16x3":
            _CACHE[key] = build_kernel_fp16x3(d, th, has_bias)
        elif MODE == "hi16lo8":
            _CACHE[key] = build_kernel_hi16lo8(d, th, has_bias)
        else:
            _CACHE[key] = build_kernel_fp32hp(d, th, has_bias)
    nc = _CACHE[key]

    in_maps = []
    if MODE == "hi16lo8":
        import ml_dtypes

        FP8NP = ml_dtypes.float8_e4m3
        W16m = (W * np.float32(2.0**6)).astype(np.float16)
        W11 = W16m.astype(np.float32) * np.float32(2.0**-6)
        Wl = W - W11
        w16_l = _wt_layout(W16m)
        a1_l = _wt_layout((Wl * np.float32(2.0**17)).astype(FP8NP))
        b2_l = _wt_layout(W16m.astype(np.float32).astype(FP8NP))
        w8_l = np.ascontiguousarray(np.stack([a1_l, b2_l], axis=3))
        for i in range(N_CORES):
            xs_i = x[:, i * B_LOC : (i + 1) * B_LOC, :].reshape(TB, CI)
            X16m = (xs_i * np.float32(2.0**11)).astype(np.float16)
            x11 = X16m.astype(np.float32) * np.float32(2.0**-11)
            xl = xs_i - x11
            c1_l = _xt_layout(xs_i.astype(FP8NP))
            d2_l = _xt_layout((xl * np.float32(2.0**11)).astype(FP8NP))
            m = {
                "x16": _xt_layout(X16m),
                "x8": np.ascontiguousarray(np.stack([c1_l, d2_l], axis=2)),
                "w16": w16_l,
                "w8": w8_l,
            }
            if has_bias:
                m["bias"] = np.ascontiguousarray(b.reshape(CO, 1))
            in_maps.append(m)
    elif MODE == "fp16x3":
        Wh, Wl = _split16(W * np.float32(WSCALE))
        wh_l = _wt_layout(Wh)
        wl_l = _wt_layout(Wl)
        for i in range(N_CORES):
            xs_i = x[:, i * B_LOC : (i + 1) * B_LOC, :].reshape(TB, CI)
            xh_i, xl_i = _split16(xs_i * np.float32(XSCALE))
            m = {
                "xh": _xt_layout(xh_i),
                "xl": _xt_layout(xl_i),
                "wh": wh_l,
                "wl": wl_l,
            }
            if has_bias:
                m["bias"] = np.ascontiguousarray(
                    (b * np.float32(SCALE)).reshape(CO, 1)
                )
            in_maps.append(m)
    else:
        wt_l = _wt_layout(W)
        for i in range(N_CORES):
            xs_i = x[:, i * B_LOC : (i + 1) * B_LOC, :].reshape(TB, CI)
            m = {"xt": _xt_layout(xs_i), "wt": wt_l}
            if has_bias:
                m["bias"] = np.ascontiguousarray(b.reshape(CO, 1))
            in_maps.append(m)

    res = run_bass_kernel_spmd(
        nc, in_maps, core_ids=list(range(N_CORES)), trace=TRACE
    )
    LAST_EXEC_NS = res.exec_time_ns

    # spikes come back [CO, TB]; transpose to [T, B_loc, CO] per core
    out = np.concatenate(
        [
            np.ascontiguousarray(r["spkT"].T).reshape(T, B_LOC, CO)
            for r in res.results
        ],
        axis=1,
    )
    return np.ascontiguousarray(out)

